# revision 23
# baseline (speedup 1.0000x reference)
"""Trainium2 Bass kernel for nn_DRNN (tree double-LSTM decoder + logits/log_softmax).

Strategy:
  - Pure data parallel: batch B=128 sharded 16 rows/core over 8 cores.
  - The T=40 recurrence is restructured:
      * ancestral LSTM: nodes processed by tree depth with father h/c gathered
        from the previous level's SBUF tiles via host-baked 0/1 selection
        matmuls; x-side projections (embed @ wih.T + biases) are hoisted out
        of the loop, computed batched, and staged through DRAM in bf16.
      * fraternal (sibling) LSTM: resets every 3 steps, so it collapses to a
        host-computed constant state + 2 batched rounds over 13 chains x 16
        rows.
  - Everything runs in bf16 (tolerance is 2e-2): weights, states,
    projections; PSUM accumulation stays fp32. logit_w is fp8e4m3.
  - Engine balance: PSUM+x gate assembly alternates DVE/GPSIMD, activations
    on ACT, gate matmuls grouped q-outer to amortize LDWEIGHTS.
  - pred head and the big logits matmul + log_softmax run batched over all
    640 (b, t) rows per core; logit_w is SBUF-resident (loaded during the
    recurrence), the logits loop is m-outer so the log_softmax tail and OUT
    DMAs overlap the next chunk's matmuls; OUT is written fp16.
"""

import sys

sys.path.insert(0, "/opt/trn_rl_repo")

import numpy as np
import ml_dtypes

import concourse.bass as bass
import concourse.bacc as bacc
import concourse.tile as tile
from concourse import mybir
from concourse import bass_utils
from concourse.masks import make_identity

F32 = mybir.dt.float32
F16 = mybir.dt.float16
F8 = mybir.dt.float8e4
BF16 = mybir.dt.bfloat16
I32 = mybir.dt.int32
AF = mybir.ActivationFunctionType
OP = mybir.AluOpType

B, T, E, H, V, FC = 128, 40, 512, 512, 10000, 2048
NC_, BC = 8, 16          # cores, batch per core
NR = BC * T              # 640 rows per core
G = 4 * H                # 2048 gate dim
NV = 20                  # logits column chunks
VC = V // NV             # 500 cols per chunk
DUMP = NR                # dump row index in HC/HF

LAST_RESULTS = None
LAST_EXEC_NS = None
SKIP_PRED = False
SKIP_LOGITS = False


def _levels(fa):
    L = np.zeros((B, T), dtype=np.int32)
    rows = np.arange(B)
    for i in range(1, T):
        L[:, i] = 1 + L[rows, fa[:, i]]
    return L


def _chunks(n):
    out = []
    o = 0
    while o < n:
        out.append((o, min(128, n - o)))
        o += 128
    return out


def _build(NL, OL, XPAD, MCH_A):
    """Build the (SPMD-common) bass program. NL: common level sizes."""
    nc = bacc.Bacc("TRN2", target_bir_lowering=False, debug=True)

    dt_in = {}

    def din(name, shape, dt):
        t = nc.dram_tensor(name, list(shape), dt, kind="ExternalInput")
        dt_in[name] = t
        return t

    # pieces of the level schedule: (level, global_off, count)
    pieces = []
    for l in range(len(NL)):
        for (o, c) in _chunks(NL[l]):
            pieces.append((l + 1, OL[l] + o, c))
    NP = len(pieces)

    emb_a = din("emb_a", [4, 128, MCH_A * 128], F8)
    emb_f = din("emb_f", [4, 128, 512], F8)
    fcT = din("fcT", [16, 128, BC], F8)
    fc_wT = din("fc_wT", [16, 128, H], F8)
    wih_a = din("wih_a", [4, 128, G], F8)
    wih_f = din("wih_f", [4, 128, G], F8)
    whh_a = din("whh_a", [4, 128, G], F8)
    whh_f = din("whh_f", [4, 128, G], F8)
    pred_wT = din("pred_wT", [8, 128, H], F8)
    lwT = din("lwT", [4, 128, V], F8)
    bias_a = din("bias_a", [1, G], BF16)
    bias_f = din("bias_f", [1, G], BF16)
    logit_b = din("logit_b", [1, V], F8)
    hf0cf0 = din("hf0cf0", [1, 2 * H], BF16)   # host-computed fraternal const state
    w0f = din("w0f", [1, G], BF16)             # hf0 @ whh_f.T (host)
    fpb = din("fpb", [128, 8], F32)            # fc_b | pred_b, per-partition cols
    gsall = din("gsall", [128, NP + 6], I32)   # gsa | gsf1 | gsf2 | gshf0 scatter rows
    NLV = len(NL)
    KPREV = [1] + [len(_chunks(NL[l])) for l in range(NLV - 1)]  # prev-level pieces
    NLP = [-(-n // 4) * 4 for n in NL]
    sels = [din(f"sel_{l + 1}", [KPREV[l], 128, NLP[l]], BF16) for l in range(NLV)]

    OUT = nc.dram_tensor("OUT", [NR, V], F16, kind="ExternalOutput")

    with tile.TileContext(nc) as tc:
        with tc.tile_pool(name="p0", bufs=1) as p0, \
             tc.tile_pool(name="dram", bufs=1, space="DRAM") as pd, \
             tc.tile_pool(name="psg", bufs=4, space="PSUM") as psg, \
             tc.tile_pool(name="pst", bufs=2, space="PSUM") as pst:

            HC = pd.tile([NR + 1, H], BF16)        # h ancestral, natural rows
            HF = pd.tile([NR + 1, H], BF16)        # h fraternal, natural rows
            SPLIT = int(OL[min(2, len(NL))])    # end of level 2 in x-order
            SPLIT2 = int(OL[min(4, len(NL))])   # end of level 4 in x-order
            XA1 = pd.tile([-(-SPLIT // 4) * 4 + 4, G], BF16)            # levels 1-2
            XA2 = pd.tile([-(-(SPLIT2 - SPLIT) // 4) * 4 + 4, G], BF16)  # levels 3-4
            XA3 = pd.tile([-(-(XPAD - SPLIT2) // 4) * 4 + 4, G], BF16)   # levels 5+
            XF1 = pd.tile([256, G], BF16)          # x-proj, sibling round 1
            XF2 = pd.tile([256, G], BF16)          # x-proj, sibling round 2

            fpb_t = p0.tile([128, 8], F32)
            nc.sync.dma_start(fpb_t[:], fpb[:])
            gsall_t = p0.tile([128, NP + 6], I32)
            nc.sync.dma_start(gsall_t[:], gsall[:])
            ident = p0.tile([128, 128], BF16)
            make_identity(nc, ident[:])
            ones_t = p0.tile([1, 128], BF16)
            nc.vector.memset(ones_t[:], 1.0)

            # persistent mid-size tiles
            xa0T = p0.tile([128, 4, BC], BF16)     # transposed fc projection
            hf0 = p0.tile([128, H], BF16)          # broadcast const states
            cf0 = p0.tile([128, H], BF16)
            biasa_bc = p0.tile([128, G], BF16)     # broadcast gate biases
            biasf_bc = p0.tile([128, G], BF16)
            lwT_t = p0.tile([128, 4, V], F8)       # logits weight, SBUF resident
            pred_wT_t = p0.tile([128, 8, H], F8)
            hc20 = p0.tile([BC, 2 * H], BF16)

            bias_a_t = p0.tile([1, G], BF16)
            nc.sync.dma_start(bias_a_t[:], bias_a[:])
            logit_b_t = p0.tile([1, V], F8)
            nc.sync.dma_start(logit_b_t[:], logit_b[:])
            bias_f_t = p0.tile([1, G], BF16)
            nc.sync.dma_start(bias_f_t[:], bias_f[:])
            w0f_t = p0.tile([1, G], BF16)
            nc.sync.dma_start(w0f_t[:], w0f[:])
            hf0cf0_t = p0.tile([1, 2 * H], BF16)
            nc.sync.dma_start(hf0cf0_t[:], hf0cf0[:])

            # broadcast gate biases and hf0/cf0 to 128 rows via ones outer-product
            bjobs = [(biasa_bc, bias_a_t, 0, G), (biasf_bc, bias_f_t, 0, G),
                     (hf0, hf0cf0_t, 0, H), (cf0, hf0cf0_t, H, H)]
            k = 0
            for (dst, src_row, off, width) in bjobs:
                for n in range(width // 512):
                    pg = psg.tile([128, 512], F32, space="PSUM", tag="pg")
                    nc.tensor.matmul(pg[:, :], ones_t[:1, :128],
                                     src_row[:1, off + n * 512:off + (n + 1) * 512],
                                     start=True, stop=True)
                    if k % 2 == 0:
                        nc.vector.tensor_copy(dst[:, n * 512:(n + 1) * 512], pg[:, :])
                    else:
                        nc.scalar.copy(dst[:, n * 512:(n + 1) * 512], pg[:, :])
                    k += 1
            # scatter hf0 rows to HF (i=0 and reset steps); all other HF/HC
            # rows are written by the recurrence, so no zero-init needed.
            nc.gpsimd.indirect_dma_start(
                out=HF[:, :], out_offset=bass.IndirectOffsetOnAxis(ap=gsall_t[:, NP + 4:NP + 5], axis=0),
                in_=hf0[:128, :], in_offset=None)
            nc.gpsimd.indirect_dma_start(
                out=HF[:, :], out_offset=bass.IndirectOffsetOnAxis(ap=gsall_t[:96, NP + 5:NP + 6], axis=0),
                in_=hf0[:96, :], in_offset=None)

            # ---------------- fc path: x_a0T = fc_w @ fc_feats.T  ----------------
            with tc.tile_pool(name="pfc", bufs=1) as pfc:
                fcT_t = pfc.tile([128, 16, BC], F8)
                fc_wT_t = pfc.tile([128, 16, H], F8)
                nc.sync.dma_start(fcT_t[:], fcT[:].rearrange("q p n -> p q n"))
                nc.sync.dma_start(fc_wT_t[:], fc_wT[:].rearrange("q p n -> p q n"))
                for mm in range(4):
                    pp = pst.tile([128, BC], F32, space="PSUM", tag="ptr2")
                    for q in range(16):
                        nc.tensor.matmul(pp[:, :], fc_wT_t[:, q, mm * 128:(mm + 1) * 128],
                                         fcT_t[:, q, :], start=(q == 0), stop=(q == 15))
                    # x_a0T chunk [128, BC] + fc_b per-partition bias
                    nc.scalar.activation(xa0T[:, mm, :], pp[:, :], AF.Identity,
                                         bias=fpb_t[:, mm:mm + 1])

            def elementwise(pgs, c_in, hc2, pc, gact):
                """gates in 4 PSUM accumulators (i f g o) -> hc2 [pc, h|c].
                ACT reads PSUM directly; gact's g-slot is reused for i*g and
                the i-slot for tanh(c2). GPSIMD takes the SBUF-only products."""
                nc.scalar.activation(gact[:pc, 0:H], pgs[0][:pc, :], AF.Sigmoid)
                nc.scalar.activation(gact[:pc, H:2 * H], pgs[1][:pc, :], AF.Sigmoid)
                nc.scalar.activation(gact[:pc, 2 * H:3 * H], pgs[2][:pc, :], AF.Tanh)
                nc.scalar.activation(gact[:pc, 3 * H:4 * H], pgs[3][:pc, :], AF.Sigmoid)
                # c2 = f*c + i*g~   (written to hc2[:, H:2H]); i*g overwrites g in place
                nc.gpsimd.tensor_tensor(out=gact[:pc, 2 * H:3 * H], in0=gact[:pc, 0:H],
                                        in1=gact[:pc, 2 * H:3 * H], op=OP.mult)
                if c_in is not None:
                    nc.vector.tensor_tensor(out=hc2[:pc, H:2 * H], in0=gact[:pc, H:2 * H],
                                            in1=c_in[:pc, :], op=OP.mult)
                    nc.vector.tensor_tensor(out=hc2[:pc, H:2 * H], in0=hc2[:pc, H:2 * H],
                                            in1=gact[:pc, 2 * H:3 * H], op=OP.add)
                else:
                    nc.vector.tensor_copy(hc2[:pc, H:2 * H], gact[:pc, 2 * H:3 * H])
                # h2 = o * tanh(c2)  (tanh lands in the dead i-slot)
                nc.scalar.activation(gact[:pc, 0:H], hc2[:pc, H:2 * H], AF.Tanh)
                nc.gpsimd.tensor_tensor(out=hc2[:pc, 0:H], in0=gact[:pc, 3 * H:4 * H],
                                        in1=gact[:pc, 0:H], op=OP.mult)

            # recurrence/pred/logits weights: issue loads before the proj
            # compute-gated XA/XF writes so no DMA lane head-blocks them
            with tc.tile_pool(name="prec", bufs=1) as prc:
              # ---------------- projections XA / XF + bias ----------------
              with tc.tile_pool(name="pproj", bufs=1) as ppj, \
                 tc.tile_pool(name="pw1", bufs=2) as pw1:
                wih_a_t = ppj.tile([128, 4, G], F8)
                emb_a_t = ppj.tile([128, 4, MCH_A * 128], F8)
                emb_f_t = ppj.tile([128, 4, 512], F8)
                wih_f_t = ppj.tile([128, 4, G], F8)
                whh_a_t = prc.tile([128, 4, G], F8)
                whh_f_t = prc.tile([128, 4, G], F8)
                nc.sync.dma_start(wih_a_t[:], wih_a[:].rearrange("q p n -> p q n"))
                nc.sync.dma_start(emb_a_t[:], emb_a[:].rearrange("q p n -> p q n"))
                nc.sync.dma_start(whh_a_t[:], whh_a[:].rearrange("q p n -> p q n"))
                nc.sync.dma_start(emb_f_t[:], emb_f[:].rearrange("q p n -> p q n"))
                nc.sync.dma_start(wih_f_t[:], wih_f[:].rearrange("q p n -> p q n"))
                nc.sync.dma_start(whh_f_t[:], whh_f[:].rearrange("q p n -> p q n"))
                nc.sync.dma_start(pred_wT_t[:], pred_wT[:].rearrange("q p n -> p q n"))
                # lwT (5 MB fp8) split into chunks drip-fed through the level loop
                LWC = 1250
                lw_jobs = [
                    lambda off=off: nc.sync.dma_start(
                        lwT_t[:, :, off:off + LWC],
                        lwT[:, :, off:off + LWC].rearrange("q p n -> p q n"))
                    for off in range(0, V, LWC)]
                # level 0 first: only needs wih_a + xa0T, unblocks level 1 early
                pgs0 = [psg.tile([128, 512], F32, space="PSUM", tag="pg", name=f"pg0_{n}") for n in range(4)]
                for q in range(4):
                    for n in range(4):
                        nc.tensor.matmul(pgs0[n][:BC, :], xa0T[:, q, :],
                                         wih_a_t[:, q, n * 512:(n + 1) * 512],
                                         start=(q == 0), stop=False)
                for n in range(4):
                    nc.tensor.matmul(pgs0[n][:BC, :], ones_t[:1, :BC],
                                     bias_a_t[:1, n * 512:(n + 1) * 512],
                                     start=False, stop=True)
                gact0 = pw1.tile([BC, G], BF16, tag="gact", bufs=1)
                elementwise(pgs0, None, hc20, BC, gact0)
                nc.sync.dma_start(
                    HC[0:NR, :].rearrange("(b t) d -> b t d", t=T)[:, 0, :], hc20[:, 0:H])

                # projections ordered by first consumer: XA1 (levels 1-2),
                # XF1 (sibling round 1 at levels 1-2), XA2 (levels 3-4),
                # XF2 (sibling round 2 at levels 3-4), XA3 (levels 5+)
                a1 = [(r, min(128, SPLIT - r)) for r in range(0, SPLIT, 128)]
                a2 = [(r, min(128, SPLIT2 - r)) for r in range(SPLIT, SPLIT2, 128)]
                a3 = [(r, min(128, XPAD - r)) for r in range(SPLIT2, XPAD, 128)]
                def proj_chunk(psrc, w, bias_bc, dst, base, ro, rc):
                    pgs = [psg.tile([128, 512], F32, space="PSUM", tag="pg", name=f"pgj_{n}") for n in range(4)]
                    for q in range(4):
                        for n in range(4):
                            nc.tensor.matmul(pgs[n][:rc, :], psrc[:, q, ro:ro + rc],
                                             w[:, q, n * 512:(n + 1) * 512],
                                             start=(q == 0), stop=(q == 3))
                    for n in range(4):
                        xc = pw1.tile([128, 512], BF16, tag="xc")
                        nc.vector.tensor_tensor(out=xc[:rc, :], in0=pgs[n][:rc, :],
                                                in1=bias_bc[:rc, n * 512:(n + 1) * 512], op=OP.add)
                        nc.sync.dma_start(dst[ro - base:ro - base + rc, n * 512:(n + 1) * 512], xc[:rc, :])

                # XA1 + XF1 are consumed first (levels 1-2, sibling round 1):
                # emit them now; the rest is drip-fed through the level loop so
                # early levels don't queue behind the whole projection.
                for (ro, rc) in a1:
                    proj_chunk(emb_a_t, wih_a_t, biasa_bc, XA1, 0, ro, rc)
                for (ro, rc) in ((0, 128), (128, 128)):
                    proj_chunk(emb_f_t, wih_f_t, biasf_bc, XF1, 0, ro, rc)
                proj_jobs = (
                    [lambda ro=ro, rc=rc: proj_chunk(emb_a_t, wih_a_t, biasa_bc, XA2, SPLIT, ro, rc)
                     for (ro, rc) in a2] +
                    [lambda ro=ro, rc=rc: proj_chunk(emb_f_t, wih_f_t, biasf_bc, XF2, 256, ro, rc)
                     for (ro, rc) in ((256, 128), (384, 128))] +
                    [lambda ro=ro, rc=rc: proj_chunk(emb_a_t, wih_a_t, biasa_bc, XA3, SPLIT2, ro, rc)
                     for (ro, rc) in a3])

                # ---------------- ancestral levels + fraternal chains ----------------
                pw2 = pw1

                def lstm_round(pc, haT, xrow_src, c_in, whh_t, extra_bias_row, hc2):
                    """one batched LSTM round -> writes hc2 tile [pc, 2H]."""
                    pgs = [psg.tile([128, 512], F32, space="PSUM", tag="pg", name=f"pgr_{n}") for n in range(4)]
                    if haT is not None:
                        for q in range(4):
                            for n in range(4):
                                nc.tensor.matmul(pgs[n][:pc, :], haT[:, q, :pc],
                                                 whh_t[:, q, n * 512:(n + 1) * 512],
                                                 start=(q == 0), stop=False)
                    if extra_bias_row is not None:
                        for n in range(4):
                            nc.tensor.matmul(pgs[n][:pc, :], ones_t[:1, :pc],
                                             extra_bias_row[:1, n * 512:(n + 1) * 512],
                                             start=(haT is None), stop=False)
                    # x-row lands in PSUM via identity matmul (frees DVE/GPSIMD)
                    for n in range(4):
                        nc.tensor.matmul(pgs[n][:pc, :], ident[:pc, :pc],
                                         xrow_src[:pc, n * 512:(n + 1) * 512],
                                         start=False, stop=True)
                    gact = pw2.tile([128, G], BF16, tag="gact2")
                    elementwise(pgs, c_in, hc2, pc, gact)
                    return hc2

                def transpose_h(src, pc, tag):
                    """src [pc, H] -> haT tile [128, 4, pc] (bf16)"""
                    haT = pw2.tile([128, 4, 128], BF16, tag=tag)
                    for q in range(4):
                        pt = pst.tile([128, 128], BF16, space="PSUM", tag="ptrt")
                        nc.tensor.transpose(pt[:, :pc], src[:pc, q * 128:(q + 1) * 128],
                                            ident[:pc, :pc])
                        nc.vector.tensor_copy(haT[:, q, :pc], pt[:, :pc])
                    return haT

                # fraternal rounds, emitted interleaved with ancestral levels so
                # the scheduler can fill PE gather-stalls with independent work
                hf1 = []

                def frat_s1(j, o, c):
                    xf_t = pw2.tile([128, G], BF16, tag="xat", bufs=3, name=f"xf1_{j}")
                    nc.sync.dma_start(xf_t[:c, :], XF1[o:o + c, :])
                    keep = prc.tile([128, 2 * H], BF16, tag=f"hf1_{j}")
                    hc2 = lstm_round(c, None, xf_t, cf0, whh_f_t, w0f_t, hc2=keep)
                    nc.gpsimd.indirect_dma_start(
                        out=HF[:, :], out_offset=bass.IndirectOffsetOnAxis(ap=gsall_t[:c, NP + j:NP + j + 1], axis=0),
                        in_=hc2[:c, 0:H], in_offset=None)
                    hf1.append(hc2)

                def frat_s2(j, o, c):
                    xf_t = pw2.tile([128, G], BF16, tag="xat", bufs=3, name=f"xf2_{j}")
                    nc.sync.dma_start(xf_t[:c, :], XF2[o:o + c, :])
                    hfT = transpose_h(hf1[j], c, "haT")
                    keep = prc.tile([128, 2 * H], BF16, tag=f"hf2_{j}")
                    hc2 = lstm_round(c, hfT, xf_t, hf1[j][:, H:2 * H], whh_f_t, None, hc2=keep)
                    nc.gpsimd.indirect_dma_start(
                        out=HF[:, :], out_offset=bass.IndirectOffsetOnAxis(ap=gsall_t[:c, NP + 2 + j:NP + 3 + j], axis=0),
                        in_=hc2[:c, 0:H], in_offset=None)

                frat = [(frat_s1, j, o, c) for j, (o, c) in enumerate(_chunks(208))] + \
                       [(frat_s2, j, o, c) for j, (o, c) in enumerate(_chunks(208))]

                # ancestral levels: father h/c of level l live in level l-1's
                # SBUF output; gather via host-baked 0/1 selection matmuls.
                # haT comes out directly transposed (lhsT = h_prev straight).
                prev_pieces = [(hc20, BC)]
                pidx = 0
                for l in range(1, len(NL) + 1):
                    if l in (5, 6, 7, 8) and frat:
                        fn, j, o, c = frat.pop(0)
                        fn(j, o, c)
                    if lw_jobs:
                        lw_jobs.pop(0)()
                    sel_t = pw2.tile([128, len(prev_pieces), NLP[l - 1]], BF16,
                                     tag="sel", name=f"sel_t{l}")
                    nc.sync.dma_start(sel_t[:], sels[l - 1][:].rearrange("k p n -> p k n"))
                    new_pieces = []
                    for (o_lvl, pc) in _chunks(NL[l - 1]):
                        po = int(OL[l - 1]) + o_lvl
                        pcg = max(pc, 2)
                        xa_t = pw2.tile([128, G], BF16, tag="xat", bufs=3)
                        if po < SPLIT:
                            nc.sync.dma_start(xa_t[:pc, :], XA1[po:po + pc, :])
                        elif po < SPLIT2:
                            nc.sync.dma_start(xa_t[:pc, :], XA2[po - SPLIT:po - SPLIT + pc, :])
                        else:
                            nc.sync.dma_start(xa_t[:pc, :], XA3[po - SPLIT2:po - SPLIT2 + pc, :])
                        # gather haT [512, pc] and c [pc, 512] from prev level
                        haT = pw2.tile([128, 4, 128], BF16, tag="haT")
                        pcp = min(-(-pc // 4) * 4, 128)
                        for mm in range(4):
                            ph = pst.tile([128, 128], F32, space="PSUM", tag="ptr2")
                            for kj, (hrp, pck) in enumerate(prev_pieces):
                                nc.tensor.matmul(ph[:, :pcp], hrp[:pck, mm * 128:(mm + 1) * 128],
                                                 sel_t[:pck, kj, o_lvl:o_lvl + pcp],
                                                 start=(kj == 0), stop=(kj == len(prev_pieces) - 1))
                            nc.vector.tensor_copy(haT[:, mm, :pc], ph[:, :pc])
                        cg = pst.tile([128, 512], F32, space="PSUM", tag="ptr2")
                        for kj, (hrp, pck) in enumerate(prev_pieces):
                            nc.tensor.matmul(cg[:pc, :], sel_t[:pck, kj, o_lvl:o_lvl + pc],
                                             hrp[:pck, H:2 * H],
                                             start=(kj == 0), stop=(kj == len(prev_pieces) - 1))
                        hc2 = prc.tile([128, 2 * H], BF16, tag=f"hc_{l % 2}_{len(new_pieces)}")
                        if pcg > pc:
                            nc.vector.memset(hc2[:pcg, :], 0.0)
                        lstm_round(pc, haT, xa_t, cg, whh_a_t, None, hc2=hc2)
                        new_pieces.append((hc2, pc))
                        # scatter h to natural rows for the pred head
                        nc.gpsimd.indirect_dma_start(
                            out=HC[:, :], out_offset=bass.IndirectOffsetOnAxis(ap=gsall_t[:pcg, pidx:pidx + 1], axis=0),
                            in_=hc2[:pcg, 0:H], in_offset=None)
                        pidx += 1
                    prev_pieces = new_pieces
                    npop = 3 if l == 1 else 2
                    for _ in range(npop):
                        if proj_jobs:
                            proj_jobs.pop(0)()

                # any fraternal rounds / lwT chunks not consumed by the interleave
                for job in proj_jobs:
                    job()
                for fn, j, o, c in frat:
                    fn(j, o, c)
                for job in lw_jobs:
                    job()

            # ---------------- pred head (transposed): outT = tanh(predW @ cat) ----------------
            with tc.tile_pool(name="plog", bufs=1) as plo:
              outT = plo.tile([128, 4, NR], BF16)   # pred output transposed
              lb_bcast = plo.tile([128, V], BF16)
              for n in range(NV):
                  pg = psg.tile([128, 512], F32, space="PSUM", tag="pg")
                  nc.tensor.matmul(pg[:, :VC], ones_t[:1, :128],
                                   logit_b_t[:1, n * VC:(n + 1) * VC], start=True, stop=True)
                  if n % 2 == 0:
                      nc.vector.tensor_copy(lb_bcast[:, n * VC:(n + 1) * VC], pg[:, :VC])
                  else:
                      nc.scalar.copy(lb_bcast[:, n * VC:(n + 1) * VC], pg[:, :VC])
              if not SKIP_PRED:
                with tc.tile_pool(name="ppred", bufs=1) as ppr:
                    catT = ppr.tile([128, 8, NR], BF16)   # pred input transposed
                    for m in range(5):
                        for q in range(4):
                            nc.sync.dma_start(catT[:, q, m * 128:(m + 1) * 128],
                                              HC[m * 128:(m + 1) * 128, q * 128:(q + 1) * 128],
                                              transpose=True)
                            nc.sync.dma_start(catT[:, 4 + q, m * 128:(m + 1) * 128],
                                              HF[m * 128:(m + 1) * 128, q * 128:(q + 1) * 128],
                                              transpose=True)
                    for mm in range(4):
                        pga = psg.tile([128, 512], F32, space="PSUM", tag="pg")
                        pgb = psg.tile([128, 512], F32, space="PSUM", tag="pg")
                        for q in range(8):
                            nc.tensor.matmul(pga[:, :512], pred_wT_t[:, q, mm * 128:(mm + 1) * 128],
                                             catT[:, q, 0:512], start=(q == 0), stop=(q == 7))
                            nc.tensor.matmul(pgb[:, :128], pred_wT_t[:, q, mm * 128:(mm + 1) * 128],
                                             catT[:, q, 512:640], start=(q == 0), stop=(q == 7))
                        nc.scalar.activation(outT[:, mm, 0:512], pga[:, :512], AF.Tanh,
                                             bias=fpb_t[:, 4 + mm:5 + mm])
                        nc.scalar.activation(outT[:, mm, 512:640], pgb[:, :128], AF.Tanh,
                                             bias=fpb_t[:, 4 + mm:5 + mm])

              # ---------------- logits + log_softmax (m-outer, overlapped tail) ----------------
              if not SKIP_LOGITS:
                with tc.tile_pool(name="plm", bufs=2) as plm, \
                     tc.tile_pool(name="pls", bufs=3) as pls:
                    for m in range(5):
                        lgs = plm.tile([128, V], BF16, tag="lgs")
                        sums = plm.tile([128, NV], F32, tag="sums")
                        for ng in range(NV // 4):
                            pgs = [psg.tile([128, 512], F32, space="PSUM", tag="pg", name=f"pgl_{j}") for j in range(4)]
                            for q in range(4):
                                for j in range(4):
                                    n = ng * 4 + j
                                    nc.tensor.matmul(pgs[j][:, :VC], outT[:, q, m * 128:(m + 1) * 128],
                                                     lwT_t[:, q, n * VC:(n + 1) * VC],
                                                     start=(q == 0), stop=(q == 3))
                            for j in range(4):
                                n = ng * 4 + j
                                nc.vector.tensor_tensor(out=lgs[:, n * VC:(n + 1) * VC],
                                                        in0=pgs[j][:, :VC],
                                                        in1=lb_bcast[:, n * VC:(n + 1) * VC], op=OP.add)
                                esc = pls.tile([128, VC], BF16, tag="esc")
                                nc.scalar.activation(esc[:, :], lgs[:, n * VC:(n + 1) * VC],
                                                     AF.Exp, accum_out=sums[:, n:n + 1])
                        lse = plm.tile([128, 1], F32, tag="lse")
                        lse2 = plm.tile([128, 1], F32, tag="lse2")
                        nc.vector.tensor_reduce(out=lse[:, :], in_=sums[:, :],
                                                axis=mybir.AxisListType.X, op=OP.add)
                        nc.scalar.activation(lse2[:, :], lse[:, :], AF.Ln)
                        for n in range(NV):
                            oc = pls.tile([128, VC], F16, tag="oc")
                            nc.gpsimd.tensor_scalar(out=oc[:, :], in0=lgs[:, n * VC:(n + 1) * VC],
                                                    scalar1=lse2[:, :1], scalar2=None,
                                                    op0=OP.subtract)
                            nc.sync.dma_start(OUT[m * 128:(m + 1) * 128, n * VC:(n + 1) * VC], oc[:, :])

    return _fin(nc)


def _fin(nc):
    # Pin exp/ln to the combined natural_log_exp activation table: the default
    # chooser alternates exp-only / ln-only tables, paying a ~1.3us table load
    # per log_softmax chunk. Emptying the exp-only / ln-only sets (indices
    # preserved) makes the fixpoint pass pick the combined table once.
    import bass_rust as _bass_rust
    from concourse.hw_specs import get_activation_tables

    tables = []
    for name, s in get_activation_tables(nc.m.arch).items():
        if name in ("exp_and_others", "natural_log", "exp_and_friends"):
            s = set()
        tables.append((name, s))
    nc.insert_act_table_loads = lambda: _bass_rust.insert_act_table_loads(nc, tables)
    nc.finalize()
    return nc


def _sigmoid(x):
    return 1.0 / (1.0 + np.exp(-x))


def _prep(word_idx, father_idx, fc_feats, embed, fc_w, fc_b,
          a_wih, a_whh, a_bih, a_bhh, f_wih, f_whh, f_bih, f_bhh,
          pred_w, pred_b, logit_w, logit_b):
    BF = ml_dtypes.bfloat16
    wi = np.asarray(word_idx).astype(np.int64)
    fa = np.asarray(father_idx).astype(np.int64)
    fc_feats = np.asarray(fc_feats, dtype=np.float32)
    embed = np.asarray(embed, dtype=np.float32)
    L = _levels(fa)
    Lmax = int(L.max())
    NL = []
    for l in range(1, Lmax + 1):
        NL.append(max(int((L[c * BC:(c + 1) * BC] == l).sum()) for c in range(NC_)))
    OL = np.concatenate([[0], np.cumsum(NL)]).astype(int)
    XPAD = int(OL[-1])
    MCH_A = -(-XPAD // 128)

    pieces = []
    for l in range(len(NL)):
        for (o, c) in _chunks(NL[l]):
            pieces.append((l + 1, int(OL[l]) + o, c))
    NP = len(pieces)

    embT = np.ascontiguousarray(embed.T).astype(BF)    # [E, V]
    wih_aT = np.ascontiguousarray(a_wih.T, dtype=np.float32).reshape(4, 128, G).astype(ml_dtypes.float8_e4m3)
    wih_fT = np.ascontiguousarray(f_wih.T, dtype=np.float32).reshape(4, 128, G).astype(ml_dtypes.float8_e4m3)
    whh_aT = np.ascontiguousarray(a_whh.T, dtype=np.float32).reshape(4, 128, G).astype(ml_dtypes.float8_e4m3)
    whh_fT = np.ascontiguousarray(f_whh.T, dtype=np.float32).reshape(4, 128, G).astype(ml_dtypes.float8_e4m3)
    fc_wT = np.ascontiguousarray(np.asarray(fc_w, np.float32).T).reshape(16, 128, H).astype(ml_dtypes.float8_e4m3)
    pred_wT_ = np.ascontiguousarray(np.asarray(pred_w, np.float32).T).reshape(8, 128, H).astype(ml_dtypes.float8_e4m3)
    lwT_ = np.ascontiguousarray(np.asarray(logit_w, np.float32).T).reshape(4, 128, V).astype(ml_dtypes.float8_e4m3)
    bias_a_ = (np.asarray(a_bih, np.float32) + np.asarray(a_bhh, np.float32)).reshape(1, G).astype(BF)
    bias_f_vec = np.asarray(f_bih, np.float32) + np.asarray(f_bhh, np.float32)
    logit_b_ = np.asarray(logit_b, np.float32).reshape(1, V).astype(ml_dtypes.float8_e4m3)

    # fraternal constant state (x=0, h=0, c=0 cell) and its whh projection
    bi, bff, bg, bo = np.split(bias_f_vec, 4)
    c0 = _sigmoid(bi) * np.tanh(bg)
    h0 = _sigmoid(bo) * np.tanh(c0)
    hf0cf0_ = np.concatenate([h0, c0]).reshape(1, 2 * H).astype(BF)
    w0f_ = (h0 @ np.asarray(f_whh, np.float32).T).reshape(1, G).astype(BF)
    bias_f_ = bias_f_vec.reshape(1, G).astype(BF)
    fpb_ = np.zeros((128, 8), np.float32)
    fpb_[:, 0:4] = np.asarray(fc_b, np.float32).reshape(4, 128).T
    fpb_[:, 4:8] = np.asarray(pred_b, np.float32).reshape(4, 128).T

    in_maps = []
    for c in range(NC_):
        gb0 = c * BC
        # ancestral node order: by (level, b, i)
        emb_a_ = np.zeros((4, 128, MCH_A * 128), np.float32)
        gsall_ = np.full((128, NP + 6), DUMP, np.int32)
        sels_ = {}
        Lc = L[gb0:gb0 + BC]
        pos_prev = {(b, 0): b for b in range(BC)}
        for l in range(1, Lmax + 1):
            nodes = [(b, i) for b in range(BC) for i in range(1, T) if Lc[b, i] == l]
            kprev = 1 if l == 1 else len(_chunks(NL[l - 2]))
            sel = np.zeros((kprev, 128, -(-NL[l - 1] // 4) * 4), np.float32)
            pos_cur = {}
            for j, (b, i) in enumerate(nodes):
                p = int(OL[l - 1]) + j
                pos_cur[(b, i)] = j
                wa = wi[gb0 + b, fa[gb0 + b, i]]
                emb_a_[:, :, p] = embT[:, wa].reshape(4, 128)
                jp = pos_prev[(b, int(fa[gb0 + b, i]))]
                sel[jp // 128, jp % 128, j] = 1.0
                for pidx, (pl, po, pc) in enumerate(pieces):
                    if pl == l and po <= p < po + pc:
                        gsall_[p - po, pidx] = b * T + i
                        break
            sels_[f"sel_{l}"] = sel.astype(BF)
            pos_prev = pos_cur
        emb_f_ = np.zeros((4, 128, 512), np.float32)
        for b in range(BC):
            for k in range(13):
                p = b * 13 + k
                emb_f_[:, :, p] = embT[:, wi[gb0 + b, 3 * k + 1]].reshape(4, 128)
                emb_f_[:, :, 256 + p] = embT[:, wi[gb0 + b, 3 * k + 2]].reshape(4, 128)
                gsall_[p % 128, NP + p // 128] = b * T + 3 * k + 2
                gsall_[p % 128, NP + 2 + p // 128] = b * T + 3 * k + 3
        hf0_rows = [b * T + i for b in range(BC) for i in ([0] + list(range(1, T, 3)))]
        for j, r in enumerate(hf0_rows):
            gsall_[j % 128, NP + 4 + j // 128] = r
        fcT_ = np.ascontiguousarray(fc_feats[gb0:gb0 + BC].T).reshape(16, 128, BC).astype(ml_dtypes.float8_e4m3)

        in_maps.append({
            "emb_a": emb_a_.astype(ml_dtypes.float8_e4m3), "emb_f": emb_f_.astype(ml_dtypes.float8_e4m3), "fcT": fcT_,
            "fc_wT": fc_wT,
            "wih_a": wih_aT, "wih_f": wih_fT, "whh_a": whh_aT, "whh_f": whh_fT,
            "pred_wT": pred_wT_, "lwT": lwT_,
            "bias_a": bias_a_, "bias_f": bias_f_, "hf0cf0": hf0cf0_, "w0f": w0f_,
            "logit_b": logit_b_, "fpb": fpb_, "gsall": gsall_,
            **sels_,
        })
    return in_maps, NL, OL, XPAD, MCH_A


def kernel(**inputs):
    global LAST_RESULTS, LAST_EXEC_NS
    in_maps, NL, OL, XPAD, MCH_A = _prep(**inputs)
    nc = _build(NL, OL, XPAD, MCH_A)
    res = bass_utils.run_bass_kernel_spmd(nc, in_maps, core_ids=list(range(NC_)))
    LAST_RESULTS = res
    LAST_EXEC_NS = res.exec_time_ns
    outs = [res.results[c]["OUT"].reshape(BC, T, V) for c in range(NC_)]
    return np.concatenate(outs, axis=0).astype(np.float32)


# ---------------------------------------------------------------------------
# Timing helper (not used by grading): the axon NTFF profile hook is absent in
# this container, so estimate device exec time by pairing executes of this
# kernel against a trivial kernel with device-resident inputs; the axon
# dispatch overhead (~100ms, high variance) cancels in the paired difference.
def _make_runner(nc, in_maps, n_cores=NC_):
    import jax
    from jax.sharding import Mesh, PartitionSpec, NamedSharding
    from concourse import bass2jax

    bass2jax.install_neuronx_cc_hook()
    if nc.dbg_addr is not None:
        in_maps = [{**m, nc.dbg_addr.name: np.zeros((1, 2), np.uint32)} for m in in_maps]
    partition_name = nc.partition_id_tensor.name if nc.partition_id_tensor else None
    in_names, out_names, out_avals, zero_outs = [], [], [], []
    for alloc in nc.m.functions[0].allocations:
        if not isinstance(alloc, mybir.MemoryLocationSet):
            continue
        name = alloc.memorylocations[0].name
        if alloc.kind == "ExternalInput":
            if name != partition_name:
                in_names.append(name)
        elif alloc.kind == "ExternalOutput":
            out_names.append(name)
            shape = tuple(alloc.tensor_shape)
            dtype = mybir.dt.np(alloc.dtype)
            out_avals.append(jax.core.ShapedArray(shape, dtype))
            zero_outs.append(np.zeros(shape, dtype))
    n_params = len(in_names)
    all_in_names = list(in_names) + list(out_names)
    if partition_name is not None:
        all_in_names.append(partition_name)

    def _body(*args):
        operands = list(args)
        if partition_name is not None:
            operands.append(bass2jax.partition_id_tensor())
        outs = bass2jax._bass_exec_p.bind(
            *operands, out_avals=tuple(out_avals), in_names=tuple(all_in_names),
            out_names=tuple(out_names), lowering_input_output_aliases=(),
            sim_require_finite=True, sim_require_nnan=True, nc=nc)
        return tuple(outs)

    devices = jax.devices()[:n_cores]
    mesh = Mesh(np.asarray(devices), ("core",))
    in_specs = (PartitionSpec("core"),) * (n_params + len(out_names))
    out_specs = (PartitionSpec("core"),) * len(out_names)
    sharded = jax.jit(
        jax.shard_map(_body, mesh=mesh, in_specs=in_specs, out_specs=out_specs,
                      check_vma=False), keep_unused=True)
    concat_in = [np.concatenate([np.asarray(in_maps[c][nm]) for c in range(n_cores)], axis=0)
                 for nm in in_names]
    concat_zeros = [np.zeros((n_cores * z.shape[0], *z.shape[1:]), z.dtype) for z in zero_outs]
    sh = NamedSharding(mesh, PartitionSpec("core"))
    dev_args = [jax.device_put(x, sh) for x in concat_in + concat_zeros]
    return sharded, dev_args


def _trivial_nc():
    nc = bacc.Bacc("TRN2", target_bir_lowering=False, debug=True)
    x = nc.dram_tensor("x", [128, 512], F32, kind="ExternalInput")
    y = nc.dram_tensor("y", [128, 512], F32, kind="ExternalOutput")
    with tile.TileContext(nc) as tc:
        with tc.tile_pool(name="sb", bufs=2) as pool:
            t = pool.tile([128, 512], F32)
            nc.sync.dma_start(t[:], x[:])
            t2 = pool.tile([128, 512], F32)
            nc.scalar.mul(t2[:], t[:], 2.0)
            nc.sync.dma_start(y[:], t2[:])
    nc.finalize()
    im = [{"x": np.zeros((128, 512), np.float32)} for _ in range(NC_)]
    return nc, im


def bench_ns(inputs, pairs=40):
    import time
    import jax
    in_maps, NL, OL, XPAD, MCH_A = _prep(**inputs)
    nc = _build(NL, OL, XPAD, MCH_A)
    run_k, args_k = _make_runner(nc, in_maps)
    tnc, tim = _trivial_nc()
    run_t, args_t = _make_runner(tnc, tim)
    jax.block_until_ready(run_k(*args_k))
    jax.block_until_ready(run_t(*args_t))
    dk, dt = [], []
    for _ in range(pairs):
        t0 = time.perf_counter()
        jax.block_until_ready(run_t(*args_t))
        t1 = time.perf_counter()
        jax.block_until_ready(run_k(*args_k))
        t2 = time.perf_counter()
        dt.append(t1 - t0)
        dk.append(t2 - t1)
    dk, dt = np.array(dk), np.array(dt)
    est = np.median(dk) - np.median(dt)
    est_min = dk.min() - dt.min()
    return int(est * 1e9), int(est_min * 1e9)
